# revision 14
# baseline (speedup 1.0000x reference)
"""Trainium2 Bass kernel for nn_DigitalPhaser (4-stage time-varying allpass
phaser with feedback; x: [64, 240000] f32).

Architecture (v2): pure batch parallelism -- 8 lanes per core, full T on
every core, ZERO collectives (the previous AllGather cost ~77us of a 152us
kernel).  The per-sample recurrence is linear time-varying in a 5-dim
minimal state s=(h1,h2,h3,h4,u), u[t]=x[t]+fb*h4[t-1]:

    s_t = M_t s_{t-1} + c_t x_t,   y_t = h4_t + x_t.

Chunked into L=120 samples: y_c = K_c x_c + U_c s_c ; s-chain via
d_c = G_c x_c.  K/U/G vary smoothly along the compile-time LFO schedule, so
they are fitted per group of 25 chunks (3000 samples, aligned to the LFO
triangle apexes at t = 24000k) as A0 + delta_c*A1; the linear term rides a
host-precomputed delta-scaled copy of x.  One group's [128,~120] stationary
then serves 200 moving columns (25 chunks x 8 lanes), vs 64 in the old
kernel, and the weight stream drops ~4x.

State recovery is core-local: the 25-chunk group propagator has norm
~1e-5, so each group's start state S_g equals the previous group's end
summary E_{g-1} exactly; within-group chunk states come from one exact
[128,125] matmul per group (contraction = 24 chunk-d's + S_g).

All coefficients are input-independent host constants.  Only x-derived
tensors (x, delta*x) and y cross HBM at runtime plus ~1MB of tiny state
reshuffles through DRAM scratch.
"""

import os
import numpy as np
import ml_dtypes

import concourse.bass as bass
import concourse.bacc as bacc
import concourse.mybir as mybir
from concourse.tile import TileContext
from concourse.bass_utils import run_bass_kernel_spmd

SAMPLE_RATE = 48000.0
F0 = 0.5
F_MIN = 1000.0
F_MAX = 4000.0
FB = 0.7

BFULL = 64
BL = 8                      # lanes per core
T = 240000
N_CORES = 8
L = 120                     # samples per chunk
C = T // L                  # 2000 chunks
GC = 25                     # chunks per interp group (3000 smp, apex-aligned)
NG = C // GC                # 80 groups
NCOL = C * BL               # 16000 moving columns, col = c*8 + lane
GCOL = GC * BL              # 200 columns per group

MODE = os.environ.get("BASS_PHASER_MODE", "f8")   # "f8" | "bf16"


# ---------------------------------------------------------------- host math
def _compute_p(n):
    t = np.arange(n, dtype=np.float32) / np.float32(SAMPLE_RATE)
    phase = np.float32(2.0 * np.pi * F0) * t
    frac = np.mod(phase / np.float32(2.0 * np.pi), np.float32(1.0))
    tri = np.where(frac < 0.5, 4.0 * frac - 1.0, 3.0 - 4.0 * frac).astype(np.float32)
    d_min = np.float32(F_MIN * 2.0 / SAMPLE_RATE)
    d_max = np.float32(F_MAX * 2.0 / SAMPLE_RATE)
    depth = np.float32((d_max - d_min) * 0.5)
    lfo = d_min + depth * (np.float32(1.0) + tri)
    tanl = np.tan(lfo.astype(np.float32))
    p = (np.float32(1.0) - tanl) / (np.float32(1.0) + tanl)
    return p.astype(np.float64)


def _build_Mc(p):
    """5-state one-step matrices; state order (h1,h2,h3,h4,u)."""
    n = p.shape[0]
    r_u = np.zeros((n, 5)); r_u[:, 3] = FB
    c_u = np.ones(n)
    r_h1 = p[:, None] * r_u; r_h1[:, 0] += p; r_h1[:, 4] -= 1.0
    c_h1 = p * c_u
    r_h2 = p[:, None] * r_h1; r_h2[:, 1] += p; r_h2[:, 0] -= 1.0
    c_h2 = p * c_h1
    r_h3 = p[:, None] * r_h2; r_h3[:, 2] += p; r_h3[:, 1] -= 1.0
    c_h3 = p * c_h2
    r_h4 = p[:, None] * r_h3; r_h4[:, 3] += p; r_h4[:, 2] -= 1.0
    c_h4 = p * c_h3
    M = np.stack([r_h1, r_h2, r_h3, r_h4, r_u], axis=1)
    c = np.stack([c_h1, c_h2, c_h3, c_h4, c_u], axis=1)
    return M, c


def _chunk_mats(p):
    """Per-chunk K [C,L,L] (with +I wet mix), U [C,L,5], G [C,5,L], P [C,5,5]."""
    M, c = _build_Mc(p)
    Mb = M.reshape(C, L, 5, 5)
    cb = c.reshape(C, L, 5)
    Phi = np.empty((C, L, 5, 5))
    Phi[:, 0] = Mb[:, 0]
    for r in range(1, L):
        Phi[:, r] = np.einsum('cij,cjk->cik', Mb[:, r], Phi[:, r - 1])
    K = np.zeros((C, L, L))
    G = np.zeros((C, 5, L))
    Tcur = cb.copy()
    for lag in range(L):
        qmax = L - lag
        idx = np.arange(qmax)
        K[:, idx + lag, idx] = Tcur[:, :qmax, 3]
        G[:, :, L - 1 - lag] = Tcur[:, L - 1 - lag, :]
        if lag < L - 1:
            nq = qmax - 1
            Tcur[:, :nq] = np.einsum('cqij,cqj->cqi', Mb[:, lag + 1:], Tcur[:, :nq])
    K[:, np.arange(L), np.arange(L)] += 1.0
    U = Phi[:, :, 3, :].copy()
    P = Phi[:, L - 1].copy()
    return K, U, G, P


def _precompute():
    p = _compute_p(T)
    K, U, G, P = _chunk_mats(p)
    delta = -1.0 + (2 * np.arange(GC) + 1) / GC          # per chunk in group
    V = np.vander(delta, 2, increasing=True)

    def gfit(A):
        A2 = A.reshape(NG, GC, -1).transpose(1, 0, 2).reshape(GC, -1)
        cth, *_ = np.linalg.lstsq(V, A2, rcond=None)
        return cth.reshape((2, NG) + A.shape[1:])

    Kc, Uc, Gc = gfit(K), gfit(U), gfit(G)

    # exact per-group state matrices
    I5 = np.eye(5)
    XiA = np.zeros((NG, 128, 128))      # rows: (m'=0..23,s)=0:120, S=120:125
    Wst = np.zeros((NG, 128, 8))        # rows: (m'=0..23,s)=0:120, d24=120:125
    for g in range(NG):
        Pg = P[g * GC:(g + 1) * GC]
        XiT = np.zeros((GC, 5, 5)); XiT[0] = I5
        for m in range(1, GC):
            XiT[m] = Pg[m - 1] @ XiT[m - 1]
        for m in range(GC):             # out cols (m,s) = m*5+s
            acc = I5
            for mp in range(m - 1, -1, -1):       # coef of d_{mp}
                if mp < GC - 1:
                    XiA[g, mp * 5:(mp + 1) * 5, m::GC][:, 0:5] = acc.T
                acc = acc @ Pg[mp]
            XiA[g, 120:125, m::GC][:, 0:5] = XiT[m].T
        acc = I5
        for mp in range(GC - 1, -1, -1):          # E_g = sum What[mp] d_mp
            if mp == GC - 1:
                Wst[g, 120:125, 0:5] = acc.T
            else:
                Wst[g, mp * 5:(mp + 1) * 5, 0:5] = acc.T
            acc = acc @ Pg[mp]

    # KU stationaries [128, L]: rows 0:120 K^T, 120:125 U^T, 125:128 zero
    KU0 = np.zeros((NG, 128, L)); KU1 = np.zeros((NG, 128, L))
    KU0[:, 0:L] = Kc[0].transpose(0, 2, 1); KU0[:, L:L + 5] = Uc[0].transpose(0, 2, 1)
    KU1[:, 0:L] = Kc[1].transpose(0, 2, 1); KU1[:, L:L + 5] = Uc[1].transpose(0, 2, 1)
    # G stationaries [128, 16]: cols 0:8 G0^T(pad), 8:16 G1^T(pad)
    Gst = np.zeros((NG, 128, 16))
    Gst[:, 0:L, 0:5] = Gc[0].transpose(0, 2, 1)
    Gst[:, 0:L, 8:13] = Gc[1].transpose(0, 2, 1)
    # per-partition delta for s~ : row m*5+s -> delta[m]
    dP = np.zeros((128, 1))
    dP[0:125, 0] = np.tile(delta, 5)
    return dict(KU0=KU0, KU1=KU1, Gst=Gst, XiA=XiA, Wst=Wst, dP=dP,
                delta=delta)


# ---------------------------------------------------------------- device
def _build_nc(mode):
    f32 = mybir.dt.float32
    bf16 = mybir.dt.bfloat16
    xdt = bf16 if mode == "bf16" else mybir.dt.float8e4

    nc = bacc.Bacc(num_devices=N_CORES)
    Par = lambda name, shape, dt: nc.declare_dram_parameter(
        name, list(shape), dt, isOutput=False)
    xT = Par("xT", (128, NCOL), bf16)
    xtT = Par("xtT", (128, NCOL), xdt)
    KU0 = Par("KU0", (NG, 128, L), bf16)
    KU1 = Par("KU1", (NG, 128, L), xdt)
    Gst = Par("Gst", (NG, 128, 16), bf16)
    XiA = Par("XiA", (NG, 128, 128), bf16)
    Wst = Par("Wst", (NG, 128, 8), bf16)
    dP = Par("dP", (128, 1), f32)
    yT = nc.declare_dram_parameter("yT", [L, NCOL], bf16, isOutput=True)

    NSPL = 4                       # x loaded in 4 slices of 20 groups each
    SCOL = NCOL // NSPL

    with TileContext(nc) as tc:
        with (
            tc.tile_pool(name="xin", bufs=1) as xp,
            tc.tile_pool(name="wts", bufs=1) as wp,
            tc.tile_pool(name="dsb", bufs=1) as dsp,
            tc.tile_pool(name="ysb", bufs=4) as yp,
            tc.tile_pool(name="ps_d", bufs=2, space="PSUM") as ps_d,
            tc.tile_pool(name="ps_e", bufs=1, space="PSUM") as ps_e,
            tc.tile_pool(name="ps_s", bufs=1, space="PSUM") as ps_s,
            tc.tile_pool(name="ps_y", bufs=3, space="PSUM") as ps_y,
            tc.tile_pool(name="dram", bufs=1, space="DRAM") as dp,
        ):
            # ---- input streams (sync queue), in slices for pipelining
            x_sb = xp.tile([128, NCOL], bf16, tag="x")
            xt_sb = xp.tile([128, NCOL], xdt, tag="xt")
            for i in range(NSPL):
                sl = slice(i * SCOL, (i + 1) * SCOL)
                nc.sync.dma_start(out=x_sb[:, sl], in_=xT[:, sl])
                nc.sync.dma_start(out=xt_sb[:, sl], in_=xtT[:, sl])

            # ---- coefficient streams (scalar queue)
            def cload(param, cols, tag, dt):
                t = wp.tile([128, NG * cols], dt, tag=tag)
                nc.scalar.dma_start(
                    out=t[:].rearrange("p (g c) -> p g c", g=NG),
                    in_=param[:, :, :].rearrange("g p c -> p g c"))
                return t

            gst_t = cload(Gst, 16, "gst", bf16)
            ku0_t = cload(KU0, L, "ku0", bf16)
            ku1_t = cload(KU1, L, "ku1", xdt)
            xia_t = cload(XiA, 128, "xia", bf16)
            wst_t = cload(Wst, 8, "wst", bf16)
            dp_t = wp.tile([128, 1], f32, tag="dp")
            nc.scalar.dma_start(out=dp_t[:], in_=dP[:, :])

            # ---- D-pass: d_c = G0 x + G1 x~  ->  d_sb [8(s), (g,m,l)]
            d_sb = dsp.tile([8, NCOL], bf16, tag="dsb")
            for g in range(NG):
                pd = ps_d.tile([8, GCOL], f32, tag="pd")
                cs = slice(g * GCOL, (g + 1) * GCOL)
                nc.tensor.matmul(pd[:], gst_t[:, g * 16:g * 16 + 8],
                                 x_sb[:, cs], start=True, stop=False)
                nc.tensor.matmul(pd[:], gst_t[:, g * 16 + 8:g * 16 + 16],
                                 xt_sb[:, cs], start=False, stop=True)
                if g % 2 == 0:
                    nc.vector.tensor_copy(out=d_sb[:, cs], in_=pd[:])
                else:
                    nc.scalar.copy(out=d_sb[:, cs], in_=pd[:])

            # ---- reshuffle D -> dT [(m,s)=0:125, (g,l)] via HBM, 5+1 DMAs
            d_dramA = dp.tile([5, NCOL], bf16, tag="dda")
            nc.gpsimd.dma_start(out=d_dramA[:, :], in_=d_sb[0:5, :])
            d_dramB = dp.tile([125, NG * BL], bf16, tag="ddb")
            dA_v = d_dramA[:, :].rearrange("s (g m l) -> s m g l",
                                           g=NG, m=GC, l=BL)
            dB_v = d_dramB[:, :].rearrange("(m s) (g l) -> s m g l",
                                           m=GC, s=5, g=NG, l=BL)
            for s in range(5):
                nc.gpsimd.dma_start(out=dB_v[s], in_=dA_v[s])
            dT_sb = dsp.tile([128, NG * BL], bf16, tag="dT")
            nc.vector.memset(dT_sb[:, :], 0.0)
            nc.gpsimd.dma_start(out=dT_sb[0:125, :], in_=d_dramB[:, :])

            # ---- E_g = What_g . dT_g ; S_g = E_{g-1} -> dT rows 120:125
            e_sb = dsp.tile([8, (NG + 1) * BL], bf16, tag="esb")
            nc.vector.memset(e_sb[:, 0:BL], 0.0)
            for h in range(2):
                pe = ps_e.tile([8, 40 * BL], f32, tag="pe")
                for q in range(40):
                    g = h * 40 + q
                    nc.tensor.matmul(pe[:, q * BL:(q + 1) * BL],
                                     wst_t[:, g * 8:(g + 1) * 8],
                                     dT_sb[:, g * BL:(g + 1) * BL],
                                     start=True, stop=True)
                nc.vector.tensor_copy(
                    out=e_sb[:, BL + h * 40 * BL: BL + (h + 1) * 40 * BL],
                    in_=pe[:])
            nc.gpsimd.dma_start(out=dT_sb[120:125, :],
                                in_=e_sb[0:5, 0:NG * BL])

            # ---- within-group state recon; out cols (s,m)-packed
            s_sb = dsp.tile([128, NG * BL], bf16, tag="ssb")
            st_sb = dsp.tile([128, NG * BL], bf16, tag="stsb")
            for h in range(2):
                ps = ps_s.tile([128, 40 * BL], f32, tag="ps")
                for q in range(40):
                    g = h * 40 + q
                    nc.tensor.matmul(ps[:, q * BL:(q + 1) * BL],
                                     xia_t[:, g * 128:(g + 1) * 128],
                                     dT_sb[:, g * BL:(g + 1) * BL],
                                     start=True, stop=True)
                osl = slice(h * 40 * BL, (h + 1) * 40 * BL)
                nc.vector.tensor_copy(out=s_sb[:, osl], in_=ps[:])
                nc.vector.tensor_scalar(out=st_sb[:, osl], in0=ps[:],
                                        scalar1=dp_t[:, 0:1], scalar2=None,
                                        op0=mybir.AluOpType.mult)

            # ---- ship states (rows s*25+m), reorder in DRAM, inject
            sA = dp.tile([125, NG * BL], bf16, tag="sA")
            tA = dp.tile([125, NG * BL], bf16, tag="tA")
            nc.gpsimd.dma_start(out=sA[:, :], in_=s_sb[0:125, :])
            nc.gpsimd.dma_start(out=tA[:, :], in_=st_sb[0:125, :])
            sB = dp.tile([5, NCOL], bf16, tag="sB")
            tB = dp.tile([5, NCOL], xdt, tag="tB")
            sA_v = sA[:, :].rearrange("(s m) (g l) -> s m g l",
                                      s=5, m=GC, g=NG, l=BL)
            tA_v = tA[:, :].rearrange("(s m) (g l) -> s m g l",
                                      s=5, m=GC, g=NG, l=BL)
            sB_v = sB[:, :].rearrange("s (g m l) -> s m g l",
                                      g=NG, m=GC, l=BL)
            tB_v = tB[:, :].rearrange("s (g m l) -> s m g l",
                                      g=NG, m=GC, l=BL)
            for s in range(5):
                nc.gpsimd.dma_start(out=sB_v[s], in_=sA_v[s])
                nc.gpsimd.dma_start(out=tB_v[s], in_=tA_v[s])
            nc.gpsimd.dma_start(out=x_sb[120:125, :], in_=sB[:, :])
            nc.gpsimd.dma_start(out=xt_sb[120:125, :], in_=tB[:, :])

            # ---- Y-pass: y = KU0 . [x;s] + KU1 . [x~;s~], 2 groups per psum
            for b in range(NG // 2):
                py = ps_y.tile([L, 2 * GCOL], f32, tag="py")
                for j in range(2):
                    g = b * 2 + j
                    cs = slice(g * GCOL, (g + 1) * GCOL)
                    ps_sl = slice(j * GCOL, (j + 1) * GCOL)
                    nc.tensor.matmul(py[:, ps_sl], ku0_t[:, g * L:(g + 1) * L],
                                     x_sb[:, cs], start=True, stop=False)
                    nc.tensor.matmul(py[:, ps_sl], ku1_t[:, g * L:(g + 1) * L],
                                     xt_sb[:, cs], start=False, stop=True)
                yt = yp.tile([L, 2 * GCOL], bf16, tag="yt")
                if b % 2 == 0:
                    nc.vector.tensor_copy(out=yt[:], in_=py[:])
                else:
                    nc.scalar.copy(out=yt[:], in_=py[:])
                eng = nc.sync if b % 2 == 0 else nc.scalar
                eng.dma_start(out=yT[:, b * 2 * GCOL:(b + 1) * 2 * GCOL],
                              in_=yt[:])

    nc.compile()
    return nc


# ---------------------------------------------------------------- driver
_CACHE = {}


def _get_built(mode):
    if mode not in _CACHE:
        coef = _precompute()
        bfdt = ml_dtypes.bfloat16
        xdt = bfdt if mode == "bf16" else ml_dtypes.float8_e4m3fn
        base = dict(
            KU0=np.ascontiguousarray(coef['KU0'].astype(bfdt)),
            KU1=np.ascontiguousarray(coef['KU1'].astype(xdt)),
            Gst=np.ascontiguousarray(coef['Gst'].astype(bfdt)),
            XiA=np.ascontiguousarray(coef['XiA'].astype(bfdt)),
            Wst=np.ascontiguousarray(coef['Wst'].astype(bfdt)),
            dP=np.ascontiguousarray(coef['dP'].astype(np.float32)),
        )
        nc = _build_nc(mode)
        _CACHE[mode] = (nc, base, coef['delta'], xdt)
    return _CACHE[mode]


def _run(x, mode, trace=False):
    nc, base, delta, xdt = _get_built(mode)
    x = np.asarray(x, dtype=np.float32)
    dfull = np.tile(delta, NG).astype(np.float32)        # [C]
    in_maps = []
    for k in range(N_CORES):
        xb = x[k * BL:(k + 1) * BL]                      # [8, 240000]
        xc = xb.reshape(BL, C, L).transpose(2, 1, 0)     # [120, C, 8]
        xrow = np.zeros((128, NCOL), np.float32)
        xrow[0:L] = xc.reshape(L, NCOL)
        xtrow = np.zeros((128, NCOL), np.float32)
        xtrow[0:L] = (xc * dfull[None, :, None]).reshape(L, NCOL)
        m = dict(base)
        m["xT"] = np.ascontiguousarray(xrow.astype(ml_dtypes.bfloat16))
        m["xtT"] = np.ascontiguousarray(xtrow.astype(xdt))
        in_maps.append(m)
    res = run_bass_kernel_spmd(nc, in_maps, list(range(N_CORES)), trace=trace)
    y = np.empty((BFULL, T), np.float32)
    for k in range(N_CORES):
        yT = np.asarray(res.results[k]["yT"]).astype(np.float32)
        y[k * BL:(k + 1) * BL] = (yT.reshape(L, C, BL)
                                  .transpose(2, 1, 0).reshape(BL, T))
    return y, res


def kernel(x):
    y, _ = _run(x, MODE, trace=False)
    return y


def run_traced(x, mode=MODE):
    return _run(x, mode, trace=True)


# revision 15
# speedup vs baseline: 1.1923x; 1.1923x over previous
"""Trainium2 Bass kernel for nn_DigitalPhaser (4-stage time-varying allpass
phaser with feedback; x: [64, 240000] f32).

Architecture (v2): pure batch parallelism -- 8 lanes per core, full T on
every core, ZERO collectives (the previous AllGather cost ~77us of a 152us
kernel).  The per-sample recurrence is linear time-varying in a 5-dim
minimal state s=(h1,h2,h3,h4,u), u[t]=x[t]+fb*h4[t-1]:

    s_t = M_t s_{t-1} + c_t x_t,   y_t = h4_t + x_t.

Chunked into L=120 samples: y_c = K_c x_c + U_c s_c ; s-chain via
d_c = G_c x_c.  K/U/G vary smoothly along the compile-time LFO schedule, so
they are fitted per group of 25 chunks (3000 samples, aligned to the LFO
triangle apexes at t = 24000k) as A0 + delta_c*A1; the linear term rides a
host-precomputed delta-scaled copy of x.  One group's [128,~120] stationary
then serves 200 moving columns (25 chunks x 8 lanes), vs 64 in the old
kernel, and the weight stream drops ~4x.

State recovery is core-local: the 25-chunk group propagator has norm
~1e-5, so each group's start state S_g equals the previous group's end
summary E_{g-1} exactly; within-group chunk states come from one exact
[128,125] matmul per group (contraction = 24 chunk-d's + S_g).

All coefficients are input-independent host constants.  Only x-derived
tensors (x, delta*x) and y cross HBM at runtime plus ~1MB of tiny state
reshuffles through DRAM scratch.
"""

import os
import numpy as np
import ml_dtypes

import concourse.bass as bass
import concourse.bacc as bacc
import concourse.mybir as mybir
from concourse.tile import TileContext
from concourse.bass_utils import run_bass_kernel_spmd

SAMPLE_RATE = 48000.0
F0 = 0.5
F_MIN = 1000.0
F_MAX = 4000.0
FB = 0.7

BFULL = 64
BL = 8                      # lanes per core
T = 240000
N_CORES = 8
L = 120                     # samples per chunk
C = T // L                  # 2000 chunks
GC = 25                     # chunks per interp group (3000 smp, apex-aligned)
NG = C // GC                # 80 groups
NCOL = C * BL               # 16000 moving columns, col = c*8 + lane
GCOL = GC * BL              # 200 columns per group

MODE = os.environ.get("BASS_PHASER_MODE", "f8")   # "f8" | "bf16"


# ---------------------------------------------------------------- host math
def _compute_p(n):
    t = np.arange(n, dtype=np.float32) / np.float32(SAMPLE_RATE)
    phase = np.float32(2.0 * np.pi * F0) * t
    frac = np.mod(phase / np.float32(2.0 * np.pi), np.float32(1.0))
    tri = np.where(frac < 0.5, 4.0 * frac - 1.0, 3.0 - 4.0 * frac).astype(np.float32)
    d_min = np.float32(F_MIN * 2.0 / SAMPLE_RATE)
    d_max = np.float32(F_MAX * 2.0 / SAMPLE_RATE)
    depth = np.float32((d_max - d_min) * 0.5)
    lfo = d_min + depth * (np.float32(1.0) + tri)
    tanl = np.tan(lfo.astype(np.float32))
    p = (np.float32(1.0) - tanl) / (np.float32(1.0) + tanl)
    return p.astype(np.float64)


def _build_Mc(p):
    """5-state one-step matrices; state order (h1,h2,h3,h4,u)."""
    n = p.shape[0]
    r_u = np.zeros((n, 5)); r_u[:, 3] = FB
    c_u = np.ones(n)
    r_h1 = p[:, None] * r_u; r_h1[:, 0] += p; r_h1[:, 4] -= 1.0
    c_h1 = p * c_u
    r_h2 = p[:, None] * r_h1; r_h2[:, 1] += p; r_h2[:, 0] -= 1.0
    c_h2 = p * c_h1
    r_h3 = p[:, None] * r_h2; r_h3[:, 2] += p; r_h3[:, 1] -= 1.0
    c_h3 = p * c_h2
    r_h4 = p[:, None] * r_h3; r_h4[:, 3] += p; r_h4[:, 2] -= 1.0
    c_h4 = p * c_h3
    M = np.stack([r_h1, r_h2, r_h3, r_h4, r_u], axis=1)
    c = np.stack([c_h1, c_h2, c_h3, c_h4, c_u], axis=1)
    return M, c


def _chunk_mats(p):
    """Per-chunk K [C,L,L] (with +I wet mix), U [C,L,5], G [C,5,L], P [C,5,5]."""
    M, c = _build_Mc(p)
    Mb = M.reshape(C, L, 5, 5)
    cb = c.reshape(C, L, 5)
    Phi = np.empty((C, L, 5, 5))
    Phi[:, 0] = Mb[:, 0]
    for r in range(1, L):
        Phi[:, r] = np.einsum('cij,cjk->cik', Mb[:, r], Phi[:, r - 1])
    K = np.zeros((C, L, L))
    G = np.zeros((C, 5, L))
    Tcur = cb.copy()
    for lag in range(L):
        qmax = L - lag
        idx = np.arange(qmax)
        K[:, idx + lag, idx] = Tcur[:, :qmax, 3]
        G[:, :, L - 1 - lag] = Tcur[:, L - 1 - lag, :]
        if lag < L - 1:
            nq = qmax - 1
            Tcur[:, :nq] = np.einsum('cqij,cqj->cqi', Mb[:, lag + 1:], Tcur[:, :nq])
    K[:, np.arange(L), np.arange(L)] += 1.0
    U = Phi[:, :, 3, :].copy()
    P = Phi[:, L - 1].copy()
    return K, U, G, P


def _precompute():
    p = _compute_p(T)
    K, U, G, P = _chunk_mats(p)
    delta = -1.0 + (2 * np.arange(GC) + 1) / GC          # per chunk in group
    V = np.vander(delta, 2, increasing=True)

    def gfit(A):
        A2 = A.reshape(NG, GC, -1).transpose(1, 0, 2).reshape(GC, -1)
        cth, *_ = np.linalg.lstsq(V, A2, rcond=None)
        return cth.reshape((2, NG) + A.shape[1:])

    Kc, Uc, Gc = gfit(K), gfit(U), gfit(G)

    # exact per-group state matrices
    I5 = np.eye(5)
    XiA = np.zeros((NG, 128, 128))      # rows: (m'=0..23,s)=0:120, S=120:125
    Wst = np.zeros((NG, 128, 8))        # rows: (m'=0..23,s)=0:120, d24=120:125
    for g in range(NG):
        Pg = P[g * GC:(g + 1) * GC]
        XiT = np.zeros((GC, 5, 5)); XiT[0] = I5
        for m in range(1, GC):
            XiT[m] = Pg[m - 1] @ XiT[m - 1]
        for m in range(GC):             # out cols (m,s) = m*5+s
            acc = I5
            for mp in range(m - 1, -1, -1):       # coef of d_{mp}
                if mp < GC - 1:
                    XiA[g, mp * 5:(mp + 1) * 5, m::GC][:, 0:5] = acc.T
                acc = acc @ Pg[mp]
            XiA[g, 120:125, m::GC][:, 0:5] = XiT[m].T
        acc = I5
        for mp in range(GC - 1, -1, -1):          # E_g = sum What[mp] d_mp
            if mp == GC - 1:
                Wst[g, 120:125, 0:5] = acc.T
            else:
                Wst[g, mp * 5:(mp + 1) * 5, 0:5] = acc.T
            acc = acc @ Pg[mp]

    # KU stationaries [128, L]: rows 0:120 K^T, 120:125 U^T, 125:128 zero
    KU0 = np.zeros((NG, 128, L)); KU1 = np.zeros((NG, 128, L))
    KU0[:, 0:L] = Kc[0].transpose(0, 2, 1); KU0[:, L:L + 5] = Uc[0].transpose(0, 2, 1)
    KU1[:, 0:L] = Kc[1].transpose(0, 2, 1); KU1[:, L:L + 5] = Uc[1].transpose(0, 2, 1)
    # G stationaries [128, 16]: cols 0:8 G0^T(pad), 8:16 G1^T(pad)
    Gst = np.zeros((NG, 128, 16))
    Gst[:, 0:L, 0:5] = Gc[0].transpose(0, 2, 1)
    Gst[:, 0:L, 8:13] = Gc[1].transpose(0, 2, 1)
    # per-partition delta for s~ : row m*5+s -> delta[m]
    dP = np.zeros((128, 1))
    dP[0:125, 0] = np.tile(delta, 5)
    return dict(KU0=KU0, KU1=KU1, Gst=Gst, XiA=XiA, Wst=Wst, dP=dP,
                delta=delta)


# ---------------------------------------------------------------- device
def _build_nc(mode):
    f32 = mybir.dt.float32
    bf16 = mybir.dt.bfloat16
    xdt = bf16 if mode == "bf16" else mybir.dt.float8e4

    nc = bacc.Bacc(num_devices=N_CORES)
    Par = lambda name, shape, dt: nc.declare_dram_parameter(
        name, list(shape), dt, isOutput=False)
    xT = Par("xT", (128, NCOL), bf16)
    xtT = Par("xtT", (128, NCOL), xdt)
    KU0 = Par("KU0", (128, NG * L), bf16)
    KU1 = Par("KU1", (128, NG * L), xdt)
    Gst = Par("Gst", (128, NG * 16), bf16)
    XiA = Par("XiA", (128, NG * 128), bf16)
    Wst = Par("Wst", (128, NG * 8), bf16)
    dP = Par("dP", (128, 1), f32)
    yT = nc.declare_dram_parameter("yT", [L, NCOL], bf16, isOutput=True)

    NSPL = 4                       # x loaded in 4 slices of 20 groups each
    SCOL = NCOL // NSPL

    with TileContext(nc) as tc:
        with (
            tc.tile_pool(name="xin", bufs=1) as xp,
            tc.tile_pool(name="wts", bufs=1) as wp,
            tc.tile_pool(name="dsb", bufs=1) as dsp,
            tc.tile_pool(name="ysb", bufs=4) as yp,
            tc.tile_pool(name="ps_d", bufs=2, space="PSUM") as ps_d,
            tc.tile_pool(name="ps_e", bufs=1, space="PSUM") as ps_e,
            tc.tile_pool(name="ps_s", bufs=1, space="PSUM") as ps_s,
            tc.tile_pool(name="ps_y", bufs=3, space="PSUM") as ps_y,
            tc.tile_pool(name="dram", bufs=1, space="DRAM") as dp,
        ):
            # ---- input streams (sync queue), in slices for pipelining
            x_sb = xp.tile([128, NCOL], bf16, tag="x")
            xt_sb = xp.tile([128, NCOL], xdt, tag="xt")
            for i in range(NSPL):
                sl = slice(i * SCOL, (i + 1) * SCOL)
                nc.sync.dma_start(out=x_sb[:, sl], in_=xT[:, sl])
                nc.sync.dma_start(out=xt_sb[:, sl], in_=xtT[:, sl])

            # ---- coefficient streams (scalar queue)
            def cload(param, cols, tag, dt):
                t = wp.tile([128, NG * cols], dt, tag=tag)
                nc.scalar.dma_start(out=t[:], in_=param[:, :])
                return t

            gst_t = cload(Gst, 16, "gst", bf16)
            ku0_t = cload(KU0, L, "ku0", bf16)
            ku1_t = cload(KU1, L, "ku1", xdt)
            xia_t = cload(XiA, 128, "xia", bf16)
            wst_t = cload(Wst, 8, "wst", bf16)
            dp_t = wp.tile([128, 1], f32, tag="dp")
            nc.scalar.dma_start(out=dp_t[:], in_=dP[:, :])

            # ---- D-pass: d_c = G0 x + G1 x~  ->  d_sb [8(s), (g,m,l)]
            d_sb = dsp.tile([8, NCOL], bf16, tag="dsb")
            for g in range(NG):
                pd = ps_d.tile([8, GCOL], f32, tag="pd")
                cs = slice(g * GCOL, (g + 1) * GCOL)
                nc.tensor.matmul(pd[:], gst_t[:, g * 16:g * 16 + 8],
                                 x_sb[:, cs], start=True, stop=False)
                nc.tensor.matmul(pd[:], gst_t[:, g * 16 + 8:g * 16 + 16],
                                 xt_sb[:, cs], start=False, stop=True)
                if g % 2 == 0:
                    nc.vector.tensor_copy(out=d_sb[:, cs], in_=pd[:])
                else:
                    nc.scalar.copy(out=d_sb[:, cs], in_=pd[:])

            # ---- reshuffle D -> dT [(m,s)=0:125, (g,l)] via HBM, 5+1 DMAs
            d_dramA = dp.tile([5, NCOL], bf16, tag="dda")
            nc.gpsimd.dma_start(out=d_dramA[:, :], in_=d_sb[0:5, :])
            d_dramB = dp.tile([125, NG * BL], bf16, tag="ddb")
            dA_v = d_dramA[:, :].rearrange("s (g m l) -> s m g l",
                                           g=NG, m=GC, l=BL)
            dB_v = d_dramB[:, :].rearrange("(m s) (g l) -> s m g l",
                                           m=GC, s=5, g=NG, l=BL)
            for s in range(5):
                nc.sync.dma_start(out=dB_v[s], in_=dA_v[s])
            dT_sb = dsp.tile([128, NG * BL], bf16, tag="dT")
            nc.vector.memset(dT_sb[:, :], 0.0)
            nc.gpsimd.dma_start(out=dT_sb[0:125, :], in_=d_dramB[:, :])

            # ---- E_g = What_g . dT_g ; S_g = E_{g-1} -> dT rows 120:125
            e_sb = dsp.tile([8, (NG + 1) * BL], bf16, tag="esb")
            nc.vector.memset(e_sb[:, 0:BL], 0.0)
            for h in range(2):
                pe = ps_e.tile([8, 40 * BL], f32, tag="pe")
                for q in range(40):
                    g = h * 40 + q
                    nc.tensor.matmul(pe[:, q * BL:(q + 1) * BL],
                                     wst_t[:, g * 8:(g + 1) * 8],
                                     dT_sb[:, g * BL:(g + 1) * BL],
                                     start=True, stop=True)
                nc.vector.tensor_copy(
                    out=e_sb[:, BL + h * 40 * BL: BL + (h + 1) * 40 * BL],
                    in_=pe[:])
            nc.gpsimd.dma_start(out=dT_sb[120:125, :],
                                in_=e_sb[0:5, 0:NG * BL])

            # ---- within-group state recon; out cols (s,m)-packed
            s_sb = dsp.tile([128, NG * BL], bf16, tag="ssb")
            st_sb = dsp.tile([128, NG * BL], bf16, tag="stsb")
            for h in range(2):
                ps = ps_s.tile([128, 40 * BL], f32, tag="ps")
                for q in range(40):
                    g = h * 40 + q
                    nc.tensor.matmul(ps[:, q * BL:(q + 1) * BL],
                                     xia_t[:, g * 128:(g + 1) * 128],
                                     dT_sb[:, g * BL:(g + 1) * BL],
                                     start=True, stop=True)
                osl = slice(h * 40 * BL, (h + 1) * 40 * BL)
                nc.vector.tensor_copy(out=s_sb[:, osl], in_=ps[:])
                nc.vector.tensor_scalar(out=st_sb[:, osl], in0=ps[:],
                                        scalar1=dp_t[:, 0:1], scalar2=None,
                                        op0=mybir.AluOpType.mult)

            # ---- ship states (rows s*25+m), reorder in DRAM, inject
            sA = dp.tile([125, NG * BL], bf16, tag="sA")
            tA = dp.tile([125, NG * BL], xdt, tag="tA")
            nc.gpsimd.dma_start(out=sA[:, :], in_=s_sb[0:125, :])
            nc.gpsimd.dma_start(out=tA[:, :], in_=st_sb[0:125, :])
            sB = dp.tile([5, NCOL], bf16, tag="sB")
            tB = dp.tile([5, NCOL], xdt, tag="tB")
            sA_v = sA[:, :].rearrange("(s m) (g l) -> s m g l",
                                      s=5, m=GC, g=NG, l=BL)
            tA_v = tA[:, :].rearrange("(s m) (g l) -> s m g l",
                                      s=5, m=GC, g=NG, l=BL)
            sB_v = sB[:, :].rearrange("s (g m l) -> s m g l",
                                      g=NG, m=GC, l=BL)
            tB_v = tB[:, :].rearrange("s (g m l) -> s m g l",
                                      g=NG, m=GC, l=BL)
            for s in range(5):
                nc.scalar.dma_start(out=sB_v[s], in_=sA_v[s])
                nc.sync.dma_start(out=tB_v[s], in_=tA_v[s])
            nc.gpsimd.dma_start(out=x_sb[120:125, :], in_=sB[:, :])
            nc.gpsimd.dma_start(out=xt_sb[120:125, :], in_=tB[:, :])

            # ---- Y-pass: y = KU0 . [x;s] + KU1 . [x~;s~], 2 groups per psum
            for b in range(NG // 2):
                py = ps_y.tile([L, 2 * GCOL], f32, tag="py")
                for j in range(2):
                    g = b * 2 + j
                    cs = slice(g * GCOL, (g + 1) * GCOL)
                    ps_sl = slice(j * GCOL, (j + 1) * GCOL)
                    nc.tensor.matmul(py[:, ps_sl], ku0_t[:, g * L:(g + 1) * L],
                                     x_sb[:, cs], start=True, stop=False)
                    nc.tensor.matmul(py[:, ps_sl], ku1_t[:, g * L:(g + 1) * L],
                                     xt_sb[:, cs], start=False, stop=True)
                yt = yp.tile([L, 2 * GCOL], bf16, tag="yt")
                if b % 2 == 0:
                    nc.vector.tensor_copy(out=yt[:], in_=py[:])
                else:
                    nc.scalar.copy(out=yt[:], in_=py[:])
                eng = nc.sync if b % 2 == 0 else nc.scalar
                eng.dma_start(out=yT[:, b * 2 * GCOL:(b + 1) * 2 * GCOL],
                              in_=yt[:])

    nc.compile()
    return nc


# ---------------------------------------------------------------- driver
_CACHE = {}


def _get_built(mode):
    if mode not in _CACHE:
        coef = _precompute()
        bfdt = ml_dtypes.bfloat16
        xdt = bfdt if mode == "bf16" else ml_dtypes.float8_e4m3fn
        def pk(a, dt):
            g, p, c = a.shape
            return np.ascontiguousarray(
                a.transpose(1, 0, 2).reshape(p, g * c).astype(dt))
        base = dict(
            KU0=pk(coef['KU0'], bfdt),
            KU1=pk(coef['KU1'], xdt),
            Gst=pk(coef['Gst'], bfdt),
            XiA=pk(coef['XiA'], bfdt),
            Wst=pk(coef['Wst'], bfdt),
            dP=np.ascontiguousarray(coef['dP'].astype(np.float32)),
        )
        nc = _build_nc(mode)
        _CACHE[mode] = (nc, base, coef['delta'], xdt)
    return _CACHE[mode]


def _run(x, mode, trace=False):
    nc, base, delta, xdt = _get_built(mode)
    x = np.asarray(x, dtype=np.float32)
    dfull = np.tile(delta, NG).astype(np.float32)        # [C]
    in_maps = []
    for k in range(N_CORES):
        xb = x[k * BL:(k + 1) * BL]                      # [8, 240000]
        xc = xb.reshape(BL, C, L).transpose(2, 1, 0)     # [120, C, 8]
        xrow = np.zeros((128, NCOL), np.float32)
        xrow[0:L] = xc.reshape(L, NCOL)
        xtrow = np.zeros((128, NCOL), np.float32)
        xtrow[0:L] = (xc * dfull[None, :, None]).reshape(L, NCOL)
        m = dict(base)
        m["xT"] = np.ascontiguousarray(xrow.astype(ml_dtypes.bfloat16))
        m["xtT"] = np.ascontiguousarray(xtrow.astype(xdt))
        in_maps.append(m)
    res = run_bass_kernel_spmd(nc, in_maps, list(range(N_CORES)), trace=trace)
    y = np.empty((BFULL, T), np.float32)
    for k in range(N_CORES):
        yT = np.asarray(res.results[k]["yT"]).astype(np.float32)
        y[k * BL:(k + 1) * BL] = (yT.reshape(L, C, BL)
                                  .transpose(2, 1, 0).reshape(BL, T))
    return y, res


def kernel(x):
    y, _ = _run(x, MODE, trace=False)
    return y


def run_traced(x, mode=MODE):
    return _run(x, mode, trace=True)
